# revision 47
# baseline (speedup 1.0000x reference)
"""Channel-attention (single-head shared attention over channels) Trainium2 kernel.

Reference computation (per batch b, C=512 channels, N=64*64=4096 spatial):
    xf = x[b].reshape(C, N)
    q = wq[:,None]*xf ; k = wk[:,None]*xf ; v = wv[:,None]*xf
    attn = softmax(q @ k.T / sqrt(N), axis=-1)        # (C, C)
    out[b] = (attn @ v).reshape(C, H, W)

Kernel strategy (data-parallel over B across 8 cores, 2 batches/core):

  Gram in fp8 DoubleRow.  x is cast once to fp8e4 with the per-channel scale
  s_c = KAPPA*wq_c folded in (the column factor of the logits MUST live in the
  data; the row factor rides the exp's per-partition scale).  The transposed
  copy [n, c] needed to contract over n is produced on the tensor engine with
  float16-carrier transposes: two adjacent fp8 n-values ride one f16 lane, so
  a 256-deep n-block transposes in 128 PE rows.  The mandatory PSUM->SBUF copy
  deinterleaves the (c, parity) pairs into the clean [n, ktile=parity, c]
  layout the DoubleRow ISA requires; the DoubleRow gram then contracts both
  parity k-tiles (256 n) per instruction at 0.5 cycles/row.  Gram row-blocks
  dc={0,1} accumulate in gp-pool PSUM banks and dc={2,3} in op-pool banks
  (idle during the gram): batch 0 streams all four blocks lag-1 behind its
  chunk loads (the fill is DMA-bound, the PE is free); batch 1 streams
  dc={0,1} inside the mm2(0) slots and runs dc={2,3} right after, before its
  exps.

  exp produces E^T[d,c] = exp(wk_d wq_c G_dc / sqrt(N)) directly from the
  Gram PSUM (per-partition ACT scale a_d = wk_d/(KAPPA^2 wq_d sqrt(N))), and
  a per-partition DVE multiply folds wv_d in: E' = E^T * wv_d, stored f32r.
  Z_c = sum_d E^T[d,c] comes from tiny PE matmuls of the plain-f32 E^T blocks
  against a ones column; they are emitted inside the mm2 stream (after the
  first q of each cc's first unit) because they trail the exp chain and would
  otherwise stall the PE right at each mm2 phase start.

  Second matmul in float32r: lhsT = E' and rhs = the UNTOUCHED staged x
  (declared f32r end to end), so no bf16 V-cast pass exists at all.  The
  deferred softmax 1/Z_c is applied in the PSUM->SBUF output copy
  (per-partition scale), which also downcasts to bf16: the store traffic
  halves and the host upcasts.  The freed PSUM banks give the mm2 4 rotating
  output banks so back-to-back accumulation groups never wait on the
  PSUM->SBUF drain.

  Batch pipeline: batch 1's load/cast/transpose/gram is interleaved into
  batch 0's mm2 phase so the PE never starves between the two mm2 blocks.
"""

import numpy as np
import ml_dtypes

import concourse.bass as bass
import concourse.tile as tile
from concourse import mybir
from concourse.bass_utils import run_bass_kernel_spmd
from concourse.masks import make_identity

P = 128
C = 512
N = 4096
B_TOTAL = 16
N_CORES = 8
B_PER_CORE = B_TOTAL // N_CORES
CI = C // P        # 4 channel chunks
NJ = N // 256      # 16 n-blocks of 256 (one fp16-carrier transpose pair each)
KAPPA = 64.0       # global fp8 range scale
F32 = mybir.dt.float32
F32R = mybir.dt.float32r
BF16 = mybir.dt.bfloat16
FP8 = mybir.dt.float8e4
F16 = mybir.dt.float16
Exp = mybir.ActivationFunctionType.Exp
Copy = mybir.ActivationFunctionType.Copy
DR = mybir.MatmulPerfMode.DoubleRow


def _split_multiwaits(nc):
    """Workaround: this walrus build rejects instructions carrying >1 sync
    wait ("Too many sync wait commands").  Hoist all but the last wait onto
    standalone EventSemaphore instructions placed just before the owner (same
    engine, so sequencer order preserves semantics)."""
    for f in nc.m.functions:
        for blk in f.blocks:
            new_insts = []
            for ins in blk.instructions:
                si = ins.sync_info
                if si is not None and si.on_wait is not None and len(si.on_wait) > 1:
                    waits = list(si.on_wait)
                    for k, w in enumerate(waits[:-1]):
                        new_insts.append(
                            mybir.InstEventSemaphore(
                                name=f"{ins.name}_splitw{k}",
                                engine=ins.engine,
                                sync_info=mybir.SyncInfo(on_wait=[w], on_update=[]),
                            )
                        )
                    si.on_wait = [waits[-1]]
                new_insts.append(ins)
            blk.instructions[:] = new_insts


def build_kernel():
    nc = bass.Bass()
    x_in = nc.dram_tensor("x", [B_PER_CORE, C, N], F32R, kind="ExternalInput")
    # packed f32 weight columns w[p, i] = w[i*128 + p]:
    #   uc = KAPPA*wq (fp8 cast scale), av = wk/(KAPPA^2*wq*sqrt(N)) (exp
    #   scale), wvc = wv (E' scale), last column = 1.0 (Z ones-vector)
    wpack_in = nc.dram_tensor("wpack", [P, 3 * CI + 1], F32, kind="ExternalInput")
    winv_in = nc.dram_tensor("winv", [P, CI], F32R, kind="ExternalInput")
    out = nc.dram_tensor("out", [B_PER_CORE, C, N], BF16, kind="ExternalOutput")

    with tile.TileContext(nc) as tc:
        with (
            tc.tile_pool(name="singles", bufs=1) as singles,
            tc.tile_pool(name="stg", bufs=2) as stg_pool,
            tc.tile_pool(name="xq", bufs=3) as xq_pool,
            tc.tile_pool(name="xt", bufs=2) as xt_pool,
            tc.tile_pool(name="ew", bufs=2) as ew_pool,
            tc.tile_pool(name="sm", bufs=4) as sm_pool,
            tc.tile_pool(name="osb", bufs=3) as osb_pool,
            tc.tile_pool(name="rz", bufs=8) as rz_pool,
            tc.tile_pool(name="gp", bufs=2, space="PSUM") as gp_pool,
            tc.tile_pool(name="tp", bufs=2, space="PSUM") as tp_pool,
            tc.tile_pool(name="op", bufs=4, space="PSUM") as op_pool,
        ):
            stg = {}
            xt8 = {}
            ew = {}
            gps = {0: {}, 1: {}}
            rzs = {}
            xr = {
                b: x_in[b].rearrange("(i p) n -> p i n", p=P)
                for b in range(B_PER_CORE)
            }

            def emit_chunk_dma(b, n0, nw):
                nsl = slice(n0, n0 + nw)
                nc.sync.dma_start(stg[b][:, :, nsl], xr[b][:, :, nsl])

            def emit_chunk_compute(b, n0, nw, eflip, deint_dve=False):
                """Cast a staged chunk to fp8 (wq-scaled), transpose through
                fp16 carriers and deinterleave into xt8."""
                nsl = slice(n0, n0 + nw)
                xq8 = xq_pool.tile([P, CI, nw], FP8, tag="xq", name=f"xq_{b}_{n0}")
                for ci in range(CI):
                    if (ci + eflip) % 2 == 0:
                        nc.vector.tensor_scalar_mul(
                            xq8[:, ci, :], stg[b][:, ci, nsl], uc[:, ci : ci + 1]
                        )
                    else:
                        nc.scalar.activation(
                            xq8[:, ci, :],
                            stg[b][:, ci, nsl],
                            func=Copy,
                            scale=uc[:, ci : ci + 1],
                        )
                nj = nw // 256
                tp = tp_pool.tile([P, 4 * nj, P], F16, tag="tp", name=f"tp_{b}_{n0}")
                for jj in range(nj):
                    for ci in range(CI):
                        nc.tensor.transpose(
                            tp[:, jj * 4 + ci, :],
                            xq8[:, ci, jj * 256 : (jj + 1) * 256].bitcast(F16),
                            identh,
                        )
                for jj in range(nj):
                    j = n0 // 256 + jj
                    src = tp.bitcast(FP8)[:, jj * 4 : (jj + 1) * 4].rearrange(
                        "p ci (c two) -> p ci two c", two=2
                    )
                    dst = xt8[b][:, j].rearrange(
                        "p two (ci c) -> p ci two c", ci=CI
                    )
                    if deint_dve or (jj + eflip) % 2 == 0:
                        nc.vector.tensor_copy(out=dst, in_=src)
                    else:
                        nc.scalar.activation(dst, src, func=Copy)

            def emit_gram(b, dcs, js, dc_outer=False):
                # row-blocks dc=0,1 accumulate in the gp pool; dc=2,3 in
                # op-pool banks (idle during the gram), so all four blocks
                # can stream together behind the chunk loads.
                order = (
                    [(dc, j) for dc in dcs for j in js]
                    if dc_outer
                    else [(dc, j) for j in js for dc in dcs]
                )
                for dc, j in order:
                    if j == 0:
                        pool, tag = (gp_pool, "gp") if dc < 2 else (op_pool, "op")
                        gps[b][dc] = pool.tile(
                            [P, C], F32, tag=tag, name=f"gp{dc}_{b}"
                        )
                    nc.tensor.matmul(
                        gps[b][dc],
                        lhsT=xt8[b][:, j, :, dc * P : (dc + 1) * P],
                        rhs=xt8[b][:, j, :, :],
                        start=(j == 0),
                        stop=(j == NJ - 1),
                        perf_mode=DR,
                    )

            def emit_expm_dc(b, dc):
                """E'[dc-block] = exp(a_d * gp) * wv_d (f32r) + Z partial sums."""
                et = sm_pool.tile([P, C], F32, tag="et")
                nc.scalar.activation(
                    et, gps[b][dc], func=Exp, scale=av[:, dc : dc + 1]
                )
                nc.vector.tensor_scalar_mul(
                    ew[b][:, dc, :], et, wvc[:, dc : dc + 1]
                )
                ets[(b, dc)] = et

            def emit_z_cc(b, cc):
                # Z_c = sum_d E^T[d,c] from the plain-f32 E^T blocks.  Emitted
                # inside the mm2 stream (not before it): the tiny matmuls wait
                # on the exp chain and would otherwise stall the PE right at
                # the mm2 phase start.
                zpt = op_pool.tile([P, 512], F32, tag="op", name=f"zp_{b}_{cc}")
                zps[(b, cc)] = zpt[:, 0:1]
                for dc in range(CI):
                    nc.tensor.matmul(
                        zps[(b, cc)],
                        lhsT=ets[(b, dc)][:, cc * P : (cc + 1) * P],
                        rhs=wones,
                        start=(dc == 0),
                        stop=(dc == CI - 1),
                    )
                rz = rz_pool.tile([P, 1], F32, name=f"rz_{b}_{cc}")
                nc.vector.reciprocal(rz, zps[(b, cc)])
                rzs[(b, cc)] = rz

            def emit_mm2h(b, cc, h, tail=False, copies_dve=False):
                csl = slice(cc * P, (cc + 1) * P)
                osb = osb_pool.tile([P, 4, 512], BF16, tag="osb")
                for q in range(4):
                    nt = h * 4 + q
                    ntl = slice(nt * 512, (nt + 1) * 512)
                    op = op_pool.tile([P, 512], F32, tag="op", name=f"op_{b}_{cc}_{nt}")
                    for dc in range(CI):
                        nc.tensor.matmul(
                            op,
                            lhsT=ew[b][:, dc, csl],
                            rhs=stg[b][:, dc, ntl],
                            start=(dc == 0),
                            stop=(dc == CI - 1),
                        )
                    if h == 0 and q == 0:
                        emit_z_cc(b, cc)
                    # deferred softmax 1/Z + bf16 downcast in the PSUM->SBUF
                    # copy; alternate engines
                    if q % 2 == 0 and not copies_dve:
                        nc.scalar.activation(
                            osb[:, q, :], op, func=Copy, scale=rzs[(b, cc)]
                        )
                    else:
                        nc.vector.tensor_scalar_mul(osb[:, q, :], op, rzs[(b, cc)])
                    if tail:
                        nc.sync.dma_start(out[b, csl, ntl], osb[:, q, :])
                if not tail:
                    nc.scalar.dma_start(
                        out[b, csl, h * 2048 : (h + 1) * 2048], osb
                    )

            def alloc_io(b):
                stg[b] = stg_pool.tile([P, CI, N], F32R, tag="stg", name=f"stg{b}")
                xt8[b] = xt_pool.tile([P, NJ, 2, C], FP8, tag="xt", name=f"xt{b}")
                ew[b] = ew_pool.tile([P, CI, C], F32R, tag="ew", name=f"ew{b}")

            zps = {}
            ets = {}

            # ---------------- batch 0: load + gram (j-streamed) ----------------
            alloc_io(0)
            chunks0 = [(0, 256), (256, 256)] + [(k * 512, 512) for k in range(1, 8)]
            emit_chunk_dma(0, *chunks0[0])
            emit_chunk_dma(0, *chunks0[1])
            wpack = singles.tile([P, 3 * CI + 1], F32)
            winv = singles.tile([P, CI], F32R)
            uc = wpack[:, 0:CI]
            av = wpack[:, CI : 2 * CI]
            wvc = wpack[:, 2 * CI : 3 * CI]
            wones = wpack[:, 3 * CI : 3 * CI + 1]
            nc.sync.dma_start(wpack, wpack_in[:, :])
            nc.sync.dma_start(winv, winv_in[:, :])
            identh = singles.tile([P, P], F16)
            make_identity(nc, identh)

            for idx, (n0, nw) in enumerate(chunks0):
                if idx >= 2:
                    emit_chunk_dma(0, n0, nw)
                emit_chunk_compute(0, n0, nw, eflip=idx % 2)
                # lag-1-chunk gram so PE never waits on copies; all four
                # row-blocks stream during the (DMA-bound) fill
                if idx >= 1:
                    pn0, pnw = chunks0[idx - 1]
                    js = [j for j in range(pn0 // 256, (pn0 + pnw) // 256) if j < 12]
                    emit_gram(0, (0, 1, 2, 3), js)
            emit_gram(0, (0, 1, 2, 3), range(12, NJ), dc_outer=True)
            # start batch 1's loads as soon as the DMA queue drains batch 0
            alloc_io(1)
            emit_chunk_dma(1, 0, 512)
            emit_chunk_dma(1, 512, 512)
            # exps of pass A release the two gram banks for pass B; the exps
            # run on ACT while pass B streams on the PE
            emit_expm_dc(0, 0)
            emit_expm_dc(0, 1)
            emit_expm_dc(0, 2)
            emit_expm_dc(0, 3)

            # ------- middle: batch 0 mm2 interleaved with batch 1 intake -------
            # batch-1 cast/transpose/gram emitted one slot AHEAD of the mm2
            # units: the casts then outrank the older units' out-copies on
            # DVE/ACT (the out-stores have DMA slack; the casts gate the PE)
            mm2_units = [(cc, h) for cc in range(CI) for h in range(2)]
            emit_chunk_compute(1, 0, 512, eflip=0)
            for i, (cc, h) in enumerate(mm2_units):
                if i + 2 <= 7:
                    emit_chunk_dma(1, (i + 2) * 512, 512)
                if i + 1 <= 7:
                    emit_chunk_compute(1, (i + 1) * 512, 512, eflip=(i + 1) % 2)
                emit_gram(1, (0, 1), [2 * i, 2 * i + 1])
                emit_mm2h(0, cc, h)
            emit_gram(1, (2, 3), range(NJ), dc_outer=True)
            emit_expm_dc(1, 0)
            emit_expm_dc(1, 1)
            emit_expm_dc(1, 2)
            emit_expm_dc(1, 3)

            # ---------------- batch 1 mm2 + store tail ----------------
            for cc in range(CI):
                for h in range(2):
                    emit_mm2h(1, cc, h, tail=(cc == CI - 1))

    _split_multiwaits(nc)
    return nc


_NC_CACHE = None


def _get_nc():
    global _NC_CACHE
    if _NC_CACHE is None:
        _NC_CACHE = build_kernel()
    return _NC_CACHE


def make_weight_inputs(wq, wk, wv):
    wq = np.asarray(wq, np.float64)
    wk = np.asarray(wk, np.float64)
    wv = np.asarray(wv, np.float64)
    wqg = np.where(np.abs(wq) < 1e-30, 1e-30, wq)  # guard the division
    wvg = np.where(np.abs(wv) < 1e-6, 1e-6, wv)
    rn = np.sqrt(np.float64(N))
    u = KAPPA * wq                        # fp8 cast scale (column logit factor)
    a = wk / (KAPPA * KAPPA * wqg * rn)   # exp per-partition scale (row factor)
    uc = u.reshape(CI, P).T
    ac = a.reshape(CI, P).T
    wvcol = wvg.reshape(CI, P).T
    ones = np.ones((P, 1), np.float64)
    wpack = np.concatenate([uc, ac, wvcol, ones], axis=1).astype(np.float32)
    winv = (1.0 / wvg).reshape(CI, P).T.astype(np.float32).copy()
    return wpack, winv


def kernel(x: np.ndarray, wq: np.ndarray, wk: np.ndarray, wv: np.ndarray) -> np.ndarray:
    assert x.shape == (B_TOTAL, C, 64, 64) and x.dtype == np.float32
    nc = _get_nc()

    wpack, winv = make_weight_inputs(wq, wk, wv)
    xr = np.ascontiguousarray(x.reshape(B_TOTAL, C, N))
    in_maps = []
    for core in range(N_CORES):
        in_maps.append(
            {
                "x": xr[core * B_PER_CORE : (core + 1) * B_PER_CORE],
                "wpack": wpack,
                "winv": winv,
            }
        )

    res = run_bass_kernel_spmd(nc, in_maps, core_ids=list(range(N_CORES)))
    outs = [np.asarray(r["out"]).astype(np.float32) for r in res.results]
    return np.concatenate(outs, axis=0).reshape(B_TOTAL, C, 64, 64)
